# revision 7
# baseline (speedup 1.0000x reference)
"""Trainium2 Bass kernel for nn_CfCBlock (CfC block: LN -> 3-layer NCP CfC scan -> residual).

Strategy (8 NeuronCores, data-parallel over batch B=32 -> 4 examples/core):
  - Per core, phase-split the computation:
      LN(x) (+gamma/beta folded into GEMM0 weights host-side)
      GEMM0: G0 = xn @ W0_in^T  (batched over all timesteps, full PE efficiency)
      scan0: sequential recurrence for layer 0 (recurrent weights only)
      GEMM1: G1 = h0_all @ W1_in^T
      scan1: layer 1 recurrence
      GEMM2: G2 = h1_all @ W2_in^T
      scan2: layer 2 recurrence
      out = x + h2, hn = [h0(T), h1(T), h2(T)]
  - 3 gates per layer instead of 4: sigmoid(ta+tb) -> fold Ws = Wta+Wtb host-side.
  - fp16 compute (PE matmul 1 cycle/row vs 4 for fp32), fp32 PSUM accumulate.
  - Layout: feature/gate dim on partitions, (batch, time) on the free dim,
    so gating elementwise ops feed the next matmul's rhs with zero transposes.
"""

import os
import sys

for _p in ("/opt/trn_rl_repo", "/opt/pypackages"):
    if _p not in sys.path and os.path.isdir(_p):
        sys.path.append(_p)

import numpy as np

import concourse.bass as bass
import concourse.tile as tile
from concourse import bacc, mybir
from concourse.bass_utils import run_bass_kernel_spmd

# ---- problem dims ----
D_MODEL = 512
UNITS = 1024
LAYER_H = [308, 204, 512]
LAYER_IN = [D_MODEL, 308, 204]
B, L_FULL = 32, 1024
LN_EPS = 1e-5
N_CORES = 8
BL = B // N_CORES  # batch per core

# padded dims
HP = [((h + 127) // 128) * 128 for h in LAYER_H]      # [384, 256, 512]
NSL = [hp // 128 for hp in HP]                        # [3, 2, 4] slices / h-chunks
INP = [((i + 127) // 128) * 128 for i in LAYER_IN]    # [512, 384, 256]
NKIN = [i // 128 for i in INP]                        # [4, 3, 2]
H_OFF = [0, 308, 512]                                 # offsets of h_l in the 1024-wide hidden state

F16 = mybir.dt.float16
F32 = mybir.dt.float32
AF = mybir.ActivationFunctionType


def _ceil_pad(a, rows, cols):
    out = np.zeros((rows, cols), np.float32)
    out[: a.shape[0], : a.shape[1]] = a
    return out


def host_prep(inputs):
    """Fold masks/gates/LN-affine, pad, transpose into lhsT layouts; fp16 cast."""
    g = np.asarray(inputs["ln_gamma"], np.float32)
    be = np.asarray(inputs["ln_beta"], np.float32)
    win_t, wh_t, bias = [], [], []
    for l in range(3):
        m = np.asarray(inputs["masks"][l], np.float32)
        W1 = np.asarray(inputs["ff1_w"][l], np.float32) * m
        W2 = np.asarray(inputs["ff2_w"][l], np.float32) * m
        Ws = np.asarray(inputs["ta_w"][l], np.float32) + np.asarray(
            inputs["tb_w"][l], np.float32
        )
        b3 = [
            np.asarray(inputs["ff1_b"][l], np.float32).copy(),
            np.asarray(inputs["ff2_b"][l], np.float32).copy(),
            np.asarray(inputs["ta_b"][l], np.float32)
            + np.asarray(inputs["tb_b"][l], np.float32),
        ]
        in_l, h = LAYER_IN[l], LAYER_H[l]
        hp, inp = HP[l], INP[l]
        Wins = [W[:, :in_l].copy() for W in (W1, W2, Ws)]
        Whs = [W[:, in_l:].copy() for W in (W1, W2, Ws)]
        if l == 0:
            # fold LN affine: xn_eff = norm(x)*g + be  ->  W@xn_eff = (W*g)@norm + W@be
            for gi in range(3):
                b3[gi] = b3[gi] + Wins[gi] @ be
                Wins[gi] = Wins[gi] * g[None, :]
        # stack gates with per-gate row padding to hp
        Win_pad = np.zeros((3 * hp, inp), np.float32)
        Wh_pad = np.zeros((3 * hp, hp), np.float32)
        b_pad = np.zeros((3 * hp,), np.float32)
        for gi in range(3):
            Win_pad[gi * hp : gi * hp + h, :in_l] = Wins[gi]
            Wh_pad[gi * hp : gi * hp + h, :h] = Whs[gi]
            b_pad[gi * hp : gi * hp + h] = b3[gi]
        # lhsT layout: (K-chunks, 128, M=3hp)
        wt = Win_pad.T.reshape(NKIN[l], 128, 3 * hp)
        ht = Wh_pad.T.reshape(NSL[l], 128, 3 * hp)
        win_t.append(np.ascontiguousarray(wt).astype(np.float16))
        wh_t.append(np.ascontiguousarray(ht).astype(np.float16))
        # bias tile (128, 3, nSl): [p, gi, j] = b_pad[gi*hp + 128j + p]
        bt = b_pad.reshape(3, NSL[l], 128).transpose(2, 0, 1)
        bias.append(np.ascontiguousarray(bt).astype(np.float32))
    return win_t, wh_t, bias


def hx_prep(hx_shard):
    """(BL, 1024) -> per-layer transposed padded fp16 (128, nSl, BL)."""
    out = []
    for l in range(3):
        h = LAYER_H[l]
        hT = np.zeros((128, NSL[l], BL), np.float32)
        hl = hx_shard[:, H_OFF[l] : H_OFF[l] + h]  # (BL, h)
        for c in range(NSL[l]):
            n = min(128, h - 128 * c)
            hT[:n, c, :] = hl[:, 128 * c : 128 * c + n].T
        out.append(hT.astype(np.float16))
    return out


def build_nc(L):
    """Build the per-core Bass graph (SPMD, no collectives)."""
    SAMP = BL * L           # samples per core
    NSC = SAMP // 128       # 128-sample chunks (LN / output)
    NTC = L // 128          # 128-t chunks per example
    NSC512 = SAMP // 512    # 512-sample chunks (GEMM)
    NT512 = L // 512        # 512-t chunks per example

    nc = bacc.Bacc("TRN2", target_bir_lowering=False, debug=False)

    x_d = nc.dram_tensor("x", [SAMP, D_MODEL], F32, kind="ExternalInput")
    id_d = nc.dram_tensor("ident", [128, 128], F16, kind="ExternalInput")
    win_d = [
        nc.dram_tensor(f"win{l}", [NKIN[l], 128, 3 * HP[l]], F16, kind="ExternalInput")
        for l in range(3)
    ]
    wh_d = [
        nc.dram_tensor(f"wh{l}", [NSL[l], 128, 3 * HP[l]], F16, kind="ExternalInput")
        for l in range(3)
    ]
    bias_d = [
        nc.dram_tensor(f"bias{l}", [128, 3, NSL[l]], F32, kind="ExternalInput")
        for l in range(3)
    ]
    hxt_d = [
        nc.dram_tensor(f"hxt{l}", [128, NSL[l], BL], F16, kind="ExternalInput")
        for l in range(3)
    ]
    out_d = nc.dram_tensor("outx", [SAMP, D_MODEL], F32, kind="ExternalOutput")
    hn_d = nc.dram_tensor("hn", [BL, UNITS], F32, kind="ExternalOutput")

    def gemm(tc, l, win_sb, bias_sb, rhs_tiles, g_sb, psum_pool, dve):
        """G_l = rhs @ Win_l^T + b  into g_sb[128, 3, nSl, BL, L] (fp16)."""
        nK = NKIN[l]
        NCH = min(512, L)
        for gi in range(3):
            for j in range(NSL[l]):
                col0 = gi * HP[l] + 128 * j
                for b_ in range(BL):
                    for t0 in range(0, L, NCH):
                        ps = psum_pool.tile([128, NCH], F32, tag="gemm_ps", bufs=4)
                        for k in range(nK):
                            nc.tensor.matmul(
                                ps[:, :],
                                win_sb[k][:, col0 : col0 + 128],
                                rhs_tiles[k][:, b_, t0 : t0 + NCH],
                                start=(k == 0),
                                stop=(k == nK - 1),
                            )
                        dve.tensor_scalar_add(
                            g_sb[:, gi, j, b_, t0 : t0 + NCH], ps[:, :], bias_sb[:, gi, j : j + 1]
                        )

    def scan(tc, l, wh_sb, g_sb, hxt_sb, h_all, pools):
        """Sequential recurrence for layer l over t=0..L-1."""
        psum_pool, work = pools
        nS = NSL[l]
        hp = HP[l]
        for t in range(L):
            ps = psum_pool.tile([128, 3, nS, BL], F32, tag="scan_ps")
            for j in range(nS):
                for gi in range(3):
                    col0 = gi * hp + 128 * j
                    for k in range(nS):
                        rhs = (
                            hxt_sb[:, k, :]
                            if t == 0
                            else h_all[k][:, :, t - 1]
                        )
                        nc.tensor.matmul(
                            ps[:, gi, j, :],
                            wh_sb[k][:, col0 : col0 + 128],
                            rhs,
                            start=(k == 0),
                            stop=(k == nS - 1),
                        )
                # pre-act += G(t)
                nc.vector.tensor_add(
                    ps[:, :, j, :], ps[:, :, j, :], g_sb[:, :, j, :, t]
                )
                th = work.tile([128, 2, BL], F16, tag=f"th{l}")
                ti = work.tile([128, BL], F16, tag=f"ti{l}")
                dd = work.tile([128, BL], F16, tag=f"dd{l}")
                ee = work.tile([128, BL], F16, tag=f"ee{l}")
                nc.scalar.activation(th[:, :, :], ps[:, 0:2, j, :], AF.Tanh)
                nc.scalar.activation(ti[:, :], ps[:, 2, j, :], AF.Sigmoid)
                nc.vector.tensor_sub(dd[:, :], th[:, 1, :], th[:, 0, :])
                nc.vector.tensor_mul(ee[:, :], ti[:, :], dd[:, :])
                nc.vector.tensor_add(h_all[j][:, :, t], th[:, 0, :], ee[:, :])

    with tile.TileContext(nc) as tc:
        with (
            tc.tile_pool(name="keep", bufs=1) as keep,
            tc.tile_pool(name="psum", bufs=2, space="PSUM") as psum_pool,
            tc.tile_pool(name="work", bufs=3) as work,
            tc.tile_pool(name="h0p", bufs=1) as h0_pool,
            tc.tile_pool(name="h1p", bufs=1) as h1_pool,
            tc.tile_pool(name="h2p", bufs=1) as h2_pool,
        ):
            ident = keep.tile([128, 128], F16)
            nc.sync.dma_start(ident[:, :], id_d[:, :])
            eps_t = keep.tile([128, 1], F32)
            nc.vector.memset(eps_t[:, :], LN_EPS)
            hxt_sb = []
            for l in range(3):
                hx_t = keep.tile([128, NSL[l], BL], F16, tag=f"hxt{l}")
                nc.sync.dma_start(hx_t[:, :, :], hxt_d[l][:, :, :])
                hxt_sb.append(hx_t)
            bias_sb = []
            for l in range(3):
                b_t = keep.tile([128, 3, NSL[l]], F32, tag=f"bias{l}")
                nc.sync.dma_start(b_t[:, :, :], bias_d[l][:, :, :])
                bias_sb.append(b_t)

            h0_all = [h0_pool.tile([128, BL, L], F16, tag=f"h0_{c}", name=f"h0_{c}") for c in range(NSL[0])]
            h1_all = [h1_pool.tile([128, BL, L], F16, tag=f"h1_{c}", name=f"h1_{c}") for c in range(NSL[1])]
            h2_all = [h2_pool.tile([128, BL, L], F16, tag=f"h2_{c}", name=f"h2_{c}") for c in range(NSL[2])]

            # ---------- Phase 1: LN + transpose -> xnT, then GEMM0 ----------
            with tc.tile_pool(name="xnt", bufs=1) as xnt_pool:
                xnT = [
                    xnt_pool.tile([128, BL, L], F16, tag=f"xnt{d}", name=f"xnt{d}") for d in range(4)
                ]
                with tc.tile_pool(name="ln", bufs=3) as ln_pool:
                    for sc in range(NSC):
                        b_, t0 = sc // NTC, 128 * (sc % NTC)
                        xt = ln_pool.tile([128, D_MODEL], F32, tag="xt")
                        nc.sync.dma_start(
                            xt[:, :], x_d[128 * sc : 128 * sc + 128, :]
                        )
                        st = ln_pool.tile([128, 4], F32, tag="st")
                        cx = ln_pool.tile([128, D_MODEL], F32, tag="cx")
                        sq = ln_pool.tile([128, D_MODEL], F32, tag="sq")
                        xn = ln_pool.tile([128, D_MODEL], F16, tag="xn")
                        nc.vector.tensor_reduce(
                            st[:, 0:1], xt[:, :], mybir.AxisListType.X, mybir.AluOpType.add
                        )
                        nc.vector.tensor_scalar_mul(st[:, 1:2], st[:, 0:1], -1.0 / D_MODEL)
                        nc.vector.tensor_scalar_add(cx[:, :], xt[:, :], st[:, 1:2])
                        nc.vector.tensor_mul(sq[:, :], cx[:, :], cx[:, :])
                        nc.vector.tensor_reduce(
                            st[:, 2:3], sq[:, :], mybir.AxisListType.X, mybir.AluOpType.add
                        )
                        # std = sqrt(var + eps); var = sumsq/D
                        nc.scalar.activation(
                            st[:, 2:3], st[:, 2:3], AF.Sqrt,
                            bias=eps_t[:, :], scale=1.0 / D_MODEL,
                        )
                        nc.vector.reciprocal(st[:, 3:4], st[:, 2:3])
                        nc.vector.tensor_scalar_mul(xn[:, :], cx[:, :], st[:, 3:4])
                        for d in range(4):
                            pt = psum_pool.tile([128, 128], F16, tag="tr_ps")
                            nc.tensor.transpose(
                                pt[:, :], xn[:, 128 * d : 128 * d + 128], ident[:, :]
                            )
                            nc.vector.tensor_copy(
                                xnT[d][:, b_, t0 : t0 + 128], pt[:, :]
                            )

                with tc.tile_pool(name="w0", bufs=1) as w_pool, tc.tile_pool(
                    name="g0", bufs=1
                ) as g0_pool:
                    win_sb = []
                    for k in range(NKIN[0]):
                        wt = w_pool.tile([128, 3 * HP[0]], F16, tag=f"w0_{k}")
                        nc.sync.dma_start(wt[:, :], win_d[0][k, :, :])
                        win_sb.append(wt)
                    g0 = g0_pool.tile([128, 3, NSL[0], BL, L], F16)
                    gemm(tc, 0, win_sb, bias_sb[0], xnT, g0, psum_pool, nc.vector)
                    # xnT freed here; scan0 + GEMM1 follow
                    with tc.tile_pool(name="wh0", bufs=1) as wh_pool:
                        wh_sb = []
                        for k in range(NSL[0]):
                            wt = wh_pool.tile([128, 3 * HP[0]], F16, tag=f"wh0_{k}")
                            nc.sync.dma_start(wt[:, :], wh_d[0][k, :, :])
                            wh_sb.append(wt)
                        scan(tc, 0, wh_sb, g0, hxt_sb[0], h0_all, (psum_pool, work))

            # ---------- Phase 2: GEMM1, scan1 ----------
            with tc.tile_pool(name="w1", bufs=1) as w_pool, tc.tile_pool(
                name="g1", bufs=1
            ) as g1_pool:
                win_sb = []
                for k in range(NKIN[1]):
                    wt = w_pool.tile([128, 3 * HP[1]], F16, tag=f"w1_{k}")
                    nc.sync.dma_start(wt[:, :], win_d[1][k, :, :])
                    win_sb.append(wt)
                g1 = g1_pool.tile([128, 3, NSL[1], BL, L], F16)
                gemm(tc, 1, win_sb, bias_sb[1], h0_all, g1, psum_pool, nc.vector)
                with tc.tile_pool(name="wh1", bufs=1) as wh_pool:
                    wh_sb = []
                    for k in range(NSL[1]):
                        wt = wh_pool.tile([128, 3 * HP[1]], F16, tag=f"wh1_{k}")
                        nc.sync.dma_start(wt[:, :], wh_d[1][k, :, :])
                        wh_sb.append(wt)
                    scan(tc, 1, wh_sb, g1, hxt_sb[1], h1_all, (psum_pool, work))

            # ---------- Phase 3: GEMM2, scan2 ----------
            with tc.tile_pool(name="w2", bufs=1) as w_pool, tc.tile_pool(
                name="g2", bufs=1
            ) as g2_pool:
                win_sb = []
                for k in range(NKIN[2]):
                    wt = w_pool.tile([128, 3 * HP[2]], F16, tag=f"w2_{k}")
                    nc.sync.dma_start(wt[:, :], win_d[2][k, :, :])
                    win_sb.append(wt)
                g2 = g2_pool.tile([128, 3, NSL[2], BL, L], F16)
                gemm(tc, 2, win_sb, bias_sb[2], h1_all, g2, psum_pool, nc.vector)
                with tc.tile_pool(name="wh2", bufs=1) as wh_pool:
                    wh_sb = []
                    for k in range(NSL[2]):
                        wt = wh_pool.tile([128, 3 * HP[2]], F16, tag=f"wh2_{k}")
                        nc.sync.dma_start(wt[:, :], wh_d[2][k, :, :])
                        wh_sb.append(wt)
                    scan(tc, 2, wh_sb, g2, hxt_sb[2], h2_all, (psum_pool, work))

            # ---------- Phase 4: out = x + h2^T ----------
            with tc.tile_pool(name="fin", bufs=3) as fin:
                for sc in range(NSC):
                    b_, t0 = sc // NTC, 128 * (sc % NTC)
                    xt = fin.tile([128, D_MODEL], F32, tag="fxt")
                    nc.sync.dma_start(xt[:, :], x_d[128 * sc : 128 * sc + 128, :])
                    ot = fin.tile([128, D_MODEL], F32, tag="fot")
                    for d in range(4):
                        pt = psum_pool.tile([128, 128], F16, tag="tr_ps")
                        nc.tensor.transpose(
                            pt[:, :], h2_all[d][:, b_, t0 : t0 + 128], ident[:, :]
                        )
                        tmp = fin.tile([128, 128], F32, tag="ftmp")
                        nc.vector.tensor_copy(tmp[:, :], pt[:, :])
                        nc.vector.tensor_add(
                            ot[:, 128 * d : 128 * d + 128],
                            xt[:, 128 * d : 128 * d + 128],
                            tmp[:, :],
                        )
                    nc.sync.dma_start(out_d[128 * sc : 128 * sc + 128, :], ot[:, :])

                # hn = [h0(T), h1(T), h2(T)]  (B, 1024)
                h_alls = [h0_all, h1_all, h2_all]
                for l in range(3):
                    for c in range(NSL[l]):
                        n = min(128, LAYER_H[l] - 128 * c)
                        pt = psum_pool.tile([BL, 128], F16, tag="tr_ps")
                        nc.tensor.transpose(
                            pt[:, :], h_alls[l][c][:, :, L - 1], ident[:, :]
                        )
                        hs = fin.tile([BL, 128], F32, tag="hn_sb")
                        nc.vector.tensor_copy(hs[:, :], pt[:, :])
                        off = H_OFF[l] + 128 * c
                        nc.sync.dma_start(hn_d[:, off : off + n], hs[:, :n])

    nc.compile()
    return nc


_CACHE = {}


def _get_nc(L):
    if L not in _CACHE:
        _CACHE[L] = build_nc(L)
    return _CACHE[L]


def kernel(x, hx, ln_gamma, ln_beta, ff1_w, ff1_b, ff2_w, ff2_b,
           ta_w, ta_b, tb_w, tb_b, masks):
    x = np.asarray(x, np.float32)
    hx = np.asarray(hx, np.float32)
    L = x.shape[1]
    inputs = dict(
        x=x, hx=hx, ln_gamma=ln_gamma, ln_beta=ln_beta,
        ff1_w=ff1_w, ff1_b=ff1_b, ff2_w=ff2_w, ff2_b=ff2_b,
        ta_w=ta_w, ta_b=ta_b, tb_w=tb_w, tb_b=tb_b, masks=masks,
    )
    win_t, wh_t, bias = host_prep(inputs)
    ident = np.eye(128, dtype=np.float16)

    nc = _get_nc(L)

    in_maps = []
    for c in range(N_CORES):
        x_sh = np.ascontiguousarray(
            x[c * BL : (c + 1) * BL].reshape(BL * L, D_MODEL)
        )
        hxt = hx_prep(hx[c * BL : (c + 1) * BL])
        m = {"x": x_sh, "ident": ident}
        for l in range(3):
            m[f"win{l}"] = win_t[l]
            m[f"wh{l}"] = wh_t[l]
            m[f"bias{l}"] = bias[l]
            m[f"hxt{l}"] = hxt[l]
        in_maps.append(m)

    trace = os.environ.get("BASS_KERNEL_TRACE", "0") == "1"
    res = run_bass_kernel_spmd(nc, in_maps, core_ids=list(range(N_CORES)), trace=trace)
    if trace and res.exec_time_ns is not None:
        print(f"HW exec time: {res.exec_time_ns} ns")

    out = np.empty((B, L, D_MODEL), np.float32)
    hn = np.empty((B, UNITS), np.float32)
    for c in range(N_CORES):
        r = res.results[c]
        out[c * BL : (c + 1) * BL] = r["outx"].reshape(BL, L, D_MODEL)
        hn[c * BL : (c + 1) * BL] = r["hn"]
    return out, hn


# revision 10
# speedup vs baseline: 1.5922x; 1.5922x over previous
"""Trainium2 Bass kernel for nn_CfCBlock (CfC block: LN -> 3-layer NCP CfC scan -> residual).

Strategy (8 NeuronCores, data-parallel over batch B=32 -> 4 examples/core):
  - Per core, phase-split the computation:
      LN(x) (+gamma/beta folded into GEMM0 weights host-side)
      GEMM0: G0 = xn @ W0_in^T  (batched over all timesteps, full PE efficiency)
      scan0: sequential recurrence for layer 0 (recurrent weights only)
      GEMM1: G1 = h0_all @ W1_in^T
      scan1: layer 1 recurrence
      GEMM2: G2 = h1_all @ W2_in^T
      scan2: layer 2 recurrence
      out = x + h2, hn = [h0(T), h1(T), h2(T)]
  - 3 gates per layer instead of 4: sigmoid(ta+tb) -> fold Ws = Wta+Wtb host-side.
  - fp16 compute (PE matmul 1 cycle/row vs 4 for fp32), fp32 PSUM accumulate.
  - Layout: feature/gate dim on partitions, (batch, time) on the free dim,
    so gating elementwise ops feed the next matmul's rhs with zero transposes.
"""

import os
import sys

for _p in ("/opt/trn_rl_repo", "/opt/pypackages"):
    if _p not in sys.path and os.path.isdir(_p):
        sys.path.append(_p)

import numpy as np

import concourse.bass as bass
import concourse.tile as tile
from concourse import bacc, mybir
from concourse.bass_utils import run_bass_kernel_spmd

# ---- problem dims ----
D_MODEL = 512
UNITS = 1024
LAYER_H = [308, 204, 512]
LAYER_IN = [D_MODEL, 308, 204]
B, L_FULL = 32, 1024
LN_EPS = 1e-5
N_CORES = 8
BL = B // N_CORES  # batch per core

# padded dims
HP = [((h + 127) // 128) * 128 for h in LAYER_H]      # [384, 256, 512]
NSL = [hp // 128 for hp in HP]                        # [3, 2, 4] slices / h-chunks
INP = [((i + 127) // 128) * 128 for i in LAYER_IN]    # [512, 384, 256]
NKIN = [i // 128 for i in INP]                        # [4, 3, 2]
H_OFF = [0, 308, 512]                                 # offsets of h_l in the 1024-wide hidden state

F16 = mybir.dt.float16
F32 = mybir.dt.float32
AF = mybir.ActivationFunctionType


def _ceil_pad(a, rows, cols):
    out = np.zeros((rows, cols), np.float32)
    out[: a.shape[0], : a.shape[1]] = a
    return out


def host_prep(inputs):
    """Fold masks/gates/LN-affine, pad, transpose into lhsT layouts; fp16 cast."""
    g = np.asarray(inputs["ln_gamma"], np.float32)
    be = np.asarray(inputs["ln_beta"], np.float32)
    win_t, wh_t, bias = [], [], []
    for l in range(3):
        m = np.asarray(inputs["masks"][l], np.float32)
        W1 = np.asarray(inputs["ff1_w"][l], np.float32) * m
        W2 = np.asarray(inputs["ff2_w"][l], np.float32) * m
        Ws = np.asarray(inputs["ta_w"][l], np.float32) + np.asarray(
            inputs["tb_w"][l], np.float32
        )
        b3 = [
            np.asarray(inputs["ff1_b"][l], np.float32).copy(),
            np.asarray(inputs["ff2_b"][l], np.float32).copy(),
            np.asarray(inputs["ta_b"][l], np.float32)
            + np.asarray(inputs["tb_b"][l], np.float32),
        ]
        in_l, h = LAYER_IN[l], LAYER_H[l]
        hp, inp = HP[l], INP[l]
        Wins = [W[:, :in_l].copy() for W in (W1, W2, Ws)]
        Whs = [W[:, in_l:].copy() for W in (W1, W2, Ws)]
        if l == 0:
            # fold LN affine: xn_eff = norm(x)*g + be  ->  W@xn_eff = (W*g)@norm + W@be
            for gi in range(3):
                b3[gi] = b3[gi] + Wins[gi] @ be
                Wins[gi] = Wins[gi] * g[None, :]
        # stack gates with per-gate row padding to hp
        Win_pad = np.zeros((3 * hp, inp), np.float32)
        Wh_pad = np.zeros((3 * hp, hp), np.float32)
        b_pad = np.zeros((3 * hp,), np.float32)
        for gi in range(3):
            Win_pad[gi * hp : gi * hp + h, :in_l] = Wins[gi]
            Wh_pad[gi * hp : gi * hp + h, :h] = Whs[gi]
            b_pad[gi * hp : gi * hp + h] = b3[gi]
        # lhsT layout: (K-chunks, 128, M=3hp)
        wt = Win_pad.T.reshape(NKIN[l], 128, 3 * hp)
        ht = Wh_pad.T.reshape(NSL[l], 128, 3 * hp)
        win_t.append(np.ascontiguousarray(wt).astype(np.float16))
        wh_t.append(np.ascontiguousarray(ht).astype(np.float16))
        # bias tile (128, 3, nSl): [p, gi, j] = b_pad[gi*hp + 128j + p]
        bt = b_pad.reshape(3, NSL[l], 128).transpose(2, 0, 1)
        bias.append(np.ascontiguousarray(bt).astype(np.float32))
    return win_t, wh_t, bias


def hx_prep(hx_shard):
    """(BL, 1024) -> per-layer transposed padded fp16 (128, nSl, BL)."""
    out = []
    for l in range(3):
        h = LAYER_H[l]
        hT = np.zeros((128, NSL[l], BL), np.float32)
        hl = hx_shard[:, H_OFF[l] : H_OFF[l] + h]  # (BL, h)
        for c in range(NSL[l]):
            n = min(128, h - 128 * c)
            hT[:n, c, :] = hl[:, 128 * c : 128 * c + n].T
        out.append(hT.astype(np.float16))
    return out


def build_nc(L):
    """Build the per-core Bass graph (SPMD, no collectives)."""
    SAMP = BL * L           # samples per core
    NSC = SAMP // 128       # 128-sample chunks (LN / output)
    NTC = L // 128          # 128-t chunks per example
    NSC512 = SAMP // 512    # 512-sample chunks (GEMM)
    NT512 = L // 512        # 512-t chunks per example

    nc = bacc.Bacc("TRN2", target_bir_lowering=False, debug=False)

    x_d = nc.dram_tensor("x", [SAMP, D_MODEL], F32, kind="ExternalInput")
    id_d = nc.dram_tensor("ident", [128, 128], F16, kind="ExternalInput")
    win_d = [
        nc.dram_tensor(f"win{l}", [NKIN[l], 128, 3 * HP[l]], F16, kind="ExternalInput")
        for l in range(3)
    ]
    wh_d = [
        nc.dram_tensor(f"wh{l}", [NSL[l], 128, 3 * HP[l]], F16, kind="ExternalInput")
        for l in range(3)
    ]
    bias_d = [
        nc.dram_tensor(f"bias{l}", [128, 3, NSL[l]], F32, kind="ExternalInput")
        for l in range(3)
    ]
    hxt_d = [
        nc.dram_tensor(f"hxt{l}", [128, NSL[l], BL], F16, kind="ExternalInput")
        for l in range(3)
    ]
    out_d = nc.dram_tensor("outx", [SAMP, D_MODEL], F32, kind="ExternalOutput")
    hn_d = nc.dram_tensor("hn", [BL, UNITS], F32, kind="ExternalOutput")

    def gemm(tc, l, win_sb, bias_sb, rhs_tiles, g_sb, psum_pool, dve):
        """G_l = rhs @ Win_l^T + b  into g_sb[128, 3, nSl, BL, L] (fp16)."""
        nK = NKIN[l]
        NCH = min(512, L)
        for gi in range(3):
            for j in range(NSL[l]):
                col0 = gi * HP[l] + 128 * j
                for b_ in range(BL):
                    for t0 in range(0, L, NCH):
                        ps = psum_pool.tile([128, NCH], F32, tag="gemm_ps", bufs=4)
                        for k in range(nK):
                            nc.tensor.matmul(
                                ps[:, :],
                                win_sb[k][:, col0 : col0 + 128],
                                rhs_tiles[:, k, b_, t0 : t0 + NCH],
                                start=(k == 0),
                                stop=(k == nK - 1),
                            )
                        dve.tensor_scalar_add(
                            g_sb[:, gi, j, b_, t0 : t0 + NCH], ps[:, :], bias_sb[:, gi, j : j + 1]
                        )

    def scan(tc, l, wh_sb, g_sb, hxt_sb, h_all, pools):
        """Sequential recurrence for layer l over t=0..L-1.

        h_all: single tile [128, nS, BL, L].
        Two slice-groups per step: group A's gating overlaps group B's
        matmuls; at step t+1 the k-ascending accumulation consumes group A's
        h-chunks first, hiding group B's gating tail.
        """
        psum_pool, work = pools
        nS = NSL[l]
        hp = HP[l]
        groups = [(0, nS - 1), (nS - 1, nS)] if nS > 1 else [(0, nS)]
        for t in range(L):
            ps = psum_pool.tile([128, 3, nS, BL], F32, tag="scan_ps")
            for ja, jb in groups:
                for j in range(ja, jb):
                    for gi in range(3):
                        col0 = gi * hp + 128 * j
                        for k in range(nS):
                            rhs = (
                                hxt_sb[:, k, :]
                                if t == 0
                                else h_all[:, k, :, t - 1]
                            )
                            nc.tensor.matmul(
                                ps[:, gi, j, :],
                                wh_sb[k][:, col0 : col0 + 128],
                                rhs,
                                start=(k == 0),
                                stop=(k == nS - 1),
                            )
            for ja, jb in groups:
                ng = jb - ja
                # pre-act += G(t), fused across gates and this slice group
                nc.vector.tensor_add(
                    ps[:, :, ja:jb, :], ps[:, :, ja:jb, :], g_sb[:, :, ja:jb, :, t]
                )
                th = work.tile([128, 2, nS, BL], F16, tag=f"th{l}")
                ti = work.tile([128, nS, BL], F16, tag=f"ti{l}")
                dd = work.tile([128, nS, BL], F16, tag=f"dd{l}")
                ee = work.tile([128, nS, BL], F16, tag=f"ee{l}")
                nc.scalar.activation(th[:, :, ja:jb, :], ps[:, 0:2, ja:jb, :], AF.Tanh)
                nc.scalar.activation(ti[:, ja:jb, :], ps[:, 2, ja:jb, :], AF.Sigmoid)
                nc.vector.tensor_sub(
                    dd[:, ja:jb, :], th[:, 1, ja:jb, :], th[:, 0, ja:jb, :]
                )
                nc.vector.tensor_mul(ee[:, ja:jb, :], ti[:, ja:jb, :], dd[:, ja:jb, :])
                nc.vector.tensor_add(
                    h_all[:, ja:jb, :, t], th[:, 0, ja:jb, :], ee[:, ja:jb, :]
                )

    with tile.TileContext(nc) as tc:
        with (
            tc.tile_pool(name="keep", bufs=1) as keep,
            tc.tile_pool(name="psum", bufs=2, space="PSUM") as psum_pool,
            tc.tile_pool(name="work", bufs=3) as work,
            tc.tile_pool(name="h0p", bufs=1) as h0_pool,
            tc.tile_pool(name="h1p", bufs=1) as h1_pool,
            tc.tile_pool(name="h2p", bufs=1) as h2_pool,
        ):
            ident = keep.tile([128, 128], F16)
            nc.sync.dma_start(ident[:, :], id_d[:, :])
            eps_t = keep.tile([128, 1], F32)
            nc.vector.memset(eps_t[:, :], LN_EPS)
            hxt_sb = []
            for l in range(3):
                hx_t = keep.tile([128, NSL[l], BL], F16, tag=f"hxt{l}")
                nc.sync.dma_start(hx_t[:, :, :], hxt_d[l][:, :, :])
                hxt_sb.append(hx_t)
            bias_sb = []
            for l in range(3):
                b_t = keep.tile([128, 3, NSL[l]], F32, tag=f"bias{l}")
                nc.sync.dma_start(b_t[:, :, :], bias_d[l][:, :, :])
                bias_sb.append(b_t)

            h0_all = h0_pool.tile([128, NSL[0], BL, L], F16, name="h0_all")
            h1_all = h1_pool.tile([128, NSL[1], BL, L], F16, name="h1_all")
            h2_all = h2_pool.tile([128, NSL[2], BL, L], F16, name="h2_all")

            # ---------- Phase 1: LN + transpose -> xnT, then GEMM0 ----------
            with tc.tile_pool(name="xnt", bufs=1) as xnt_pool:
                xnT = xnt_pool.tile([128, 4, BL, L], F16, name="xnT")
                with tc.tile_pool(name="ln", bufs=3) as ln_pool:
                    for sc in range(NSC):
                        b_, t0 = sc // NTC, 128 * (sc % NTC)
                        xt = ln_pool.tile([128, D_MODEL], F32, tag="xt")
                        nc.sync.dma_start(
                            xt[:, :], x_d[128 * sc : 128 * sc + 128, :]
                        )
                        st = ln_pool.tile([128, 4], F32, tag="st")
                        cx = ln_pool.tile([128, D_MODEL], F32, tag="cx")
                        sq = ln_pool.tile([128, D_MODEL], F32, tag="sq")
                        xn = ln_pool.tile([128, D_MODEL], F16, tag="xn")
                        nc.vector.tensor_reduce(
                            st[:, 0:1], xt[:, :], mybir.AxisListType.X, mybir.AluOpType.add
                        )
                        nc.vector.tensor_scalar_mul(st[:, 1:2], st[:, 0:1], -1.0 / D_MODEL)
                        nc.vector.tensor_scalar_add(cx[:, :], xt[:, :], st[:, 1:2])
                        nc.vector.tensor_mul(sq[:, :], cx[:, :], cx[:, :])
                        nc.vector.tensor_reduce(
                            st[:, 2:3], sq[:, :], mybir.AxisListType.X, mybir.AluOpType.add
                        )
                        # std = sqrt(var + eps); var = sumsq/D
                        nc.scalar.activation(
                            st[:, 2:3], st[:, 2:3], AF.Sqrt,
                            bias=eps_t[:, :], scale=1.0 / D_MODEL,
                        )
                        nc.vector.reciprocal(st[:, 3:4], st[:, 2:3])
                        nc.vector.tensor_scalar_mul(xn[:, :], cx[:, :], st[:, 3:4])
                        for d in range(4):
                            pt = psum_pool.tile([128, 128], F16, tag="tr_ps")
                            nc.tensor.transpose(
                                pt[:, :], xn[:, 128 * d : 128 * d + 128], ident[:, :]
                            )
                            nc.vector.tensor_copy(
                                xnT[:, d, b_, t0 : t0 + 128], pt[:, :]
                            )

                with tc.tile_pool(name="w0", bufs=1) as w_pool, tc.tile_pool(
                    name="g0", bufs=1
                ) as g0_pool:
                    win_sb = []
                    for k in range(NKIN[0]):
                        wt = w_pool.tile([128, 3 * HP[0]], F16, tag=f"w0_{k}")
                        nc.sync.dma_start(wt[:, :], win_d[0][k, :, :])
                        win_sb.append(wt)
                    g0 = g0_pool.tile([128, 3, NSL[0], BL, L], F16)
                    gemm(tc, 0, win_sb, bias_sb[0], xnT, g0, psum_pool, nc.vector)
                    # xnT freed here; scan0 + GEMM1 follow
                    with tc.tile_pool(name="wh0", bufs=1) as wh_pool:
                        wh_sb = []
                        for k in range(NSL[0]):
                            wt = wh_pool.tile([128, 3 * HP[0]], F16, tag=f"wh0_{k}")
                            nc.sync.dma_start(wt[:, :], wh_d[0][k, :, :])
                            wh_sb.append(wt)
                        scan(tc, 0, wh_sb, g0, hxt_sb[0], h0_all, (psum_pool, work))

            # ---------- Phase 2: GEMM1, scan1 ----------
            with tc.tile_pool(name="w1", bufs=1) as w_pool, tc.tile_pool(
                name="g1", bufs=1
            ) as g1_pool:
                win_sb = []
                for k in range(NKIN[1]):
                    wt = w_pool.tile([128, 3 * HP[1]], F16, tag=f"w1_{k}")
                    nc.sync.dma_start(wt[:, :], win_d[1][k, :, :])
                    win_sb.append(wt)
                g1 = g1_pool.tile([128, 3, NSL[1], BL, L], F16)
                gemm(tc, 1, win_sb, bias_sb[1], h0_all, g1, psum_pool, nc.vector)
                with tc.tile_pool(name="wh1", bufs=1) as wh_pool:
                    wh_sb = []
                    for k in range(NSL[1]):
                        wt = wh_pool.tile([128, 3 * HP[1]], F16, tag=f"wh1_{k}")
                        nc.sync.dma_start(wt[:, :], wh_d[1][k, :, :])
                        wh_sb.append(wt)
                    scan(tc, 1, wh_sb, g1, hxt_sb[1], h1_all, (psum_pool, work))

            # ---------- Phase 3: GEMM2, scan2 ----------
            with tc.tile_pool(name="w2", bufs=1) as w_pool, tc.tile_pool(
                name="g2", bufs=1
            ) as g2_pool:
                win_sb = []
                for k in range(NKIN[2]):
                    wt = w_pool.tile([128, 3 * HP[2]], F16, tag=f"w2_{k}")
                    nc.sync.dma_start(wt[:, :], win_d[2][k, :, :])
                    win_sb.append(wt)
                g2 = g2_pool.tile([128, 3, NSL[2], BL, L], F16)
                gemm(tc, 2, win_sb, bias_sb[2], h1_all, g2, psum_pool, nc.vector)
                with tc.tile_pool(name="wh2", bufs=1) as wh_pool:
                    wh_sb = []
                    for k in range(NSL[2]):
                        wt = wh_pool.tile([128, 3 * HP[2]], F16, tag=f"wh2_{k}")
                        nc.sync.dma_start(wt[:, :], wh_d[2][k, :, :])
                        wh_sb.append(wt)
                    scan(tc, 2, wh_sb, g2, hxt_sb[2], h2_all, (psum_pool, work))

            # ---------- Phase 4: out = x + h2^T ----------
            with tc.tile_pool(name="fin", bufs=3) as fin:
                for sc in range(NSC):
                    b_, t0 = sc // NTC, 128 * (sc % NTC)
                    xt = fin.tile([128, D_MODEL], F32, tag="fxt")
                    nc.sync.dma_start(xt[:, :], x_d[128 * sc : 128 * sc + 128, :])
                    ot = fin.tile([128, D_MODEL], F32, tag="fot")
                    for d in range(4):
                        pt = psum_pool.tile([128, 128], F16, tag="tr_ps")
                        nc.tensor.transpose(
                            pt[:, :], h2_all[:, d, b_, t0 : t0 + 128], ident[:, :]
                        )
                        tmp = fin.tile([128, 128], F32, tag="ftmp")
                        nc.vector.tensor_copy(tmp[:, :], pt[:, :])
                        nc.vector.tensor_add(
                            ot[:, 128 * d : 128 * d + 128],
                            xt[:, 128 * d : 128 * d + 128],
                            tmp[:, :],
                        )
                    nc.sync.dma_start(out_d[128 * sc : 128 * sc + 128, :], ot[:, :])

                # hn = [h0(T), h1(T), h2(T)]  (B, 1024)
                h_alls = [h0_all, h1_all, h2_all]
                for l in range(3):
                    for c in range(NSL[l]):
                        n = min(128, LAYER_H[l] - 128 * c)
                        pt = psum_pool.tile([BL, 128], F16, tag="tr_ps")
                        nc.tensor.transpose(
                            pt[:, :], h_alls[l][:, c, :, L - 1], ident[:, :]
                        )
                        hs = fin.tile([BL, 128], F32, tag="hn_sb")
                        nc.vector.tensor_copy(hs[:, :], pt[:, :])
                        off = H_OFF[l] + 128 * c
                        nc.sync.dma_start(hn_d[:, off : off + n], hs[:, :n])

    nc.compile()
    return nc


_CACHE = {}


def _get_nc(L):
    if L not in _CACHE:
        _CACHE[L] = build_nc(L)
    return _CACHE[L]


def kernel(x, hx, ln_gamma, ln_beta, ff1_w, ff1_b, ff2_w, ff2_b,
           ta_w, ta_b, tb_w, tb_b, masks):
    x = np.asarray(x, np.float32)
    hx = np.asarray(hx, np.float32)
    L = x.shape[1]
    inputs = dict(
        x=x, hx=hx, ln_gamma=ln_gamma, ln_beta=ln_beta,
        ff1_w=ff1_w, ff1_b=ff1_b, ff2_w=ff2_w, ff2_b=ff2_b,
        ta_w=ta_w, ta_b=ta_b, tb_w=tb_w, tb_b=tb_b, masks=masks,
    )
    win_t, wh_t, bias = host_prep(inputs)
    ident = np.eye(128, dtype=np.float16)

    nc = _get_nc(L)

    in_maps = []
    for c in range(N_CORES):
        x_sh = np.ascontiguousarray(
            x[c * BL : (c + 1) * BL].reshape(BL * L, D_MODEL)
        )
        hxt = hx_prep(hx[c * BL : (c + 1) * BL])
        m = {"x": x_sh, "ident": ident}
        for l in range(3):
            m[f"win{l}"] = win_t[l]
            m[f"wh{l}"] = wh_t[l]
            m[f"bias{l}"] = bias[l]
            m[f"hxt{l}"] = hxt[l]
        in_maps.append(m)

    trace = os.environ.get("BASS_KERNEL_TRACE", "0") == "1"
    res = run_bass_kernel_spmd(nc, in_maps, core_ids=list(range(N_CORES)), trace=trace)
    if trace and res.exec_time_ns is not None:
        print(f"HW exec time: {res.exec_time_ns} ns")

    out = np.empty((B, L, D_MODEL), np.float32)
    hn = np.empty((B, UNITS), np.float32)
    for c in range(N_CORES):
        r = res.results[c]
        out[c * BL : (c + 1) * BL] = r["outx"].reshape(BL, L, D_MODEL)
        hn[c * BL : (c + 1) * BL] = r["hn"]
    return out, hn
